# revision 27
# baseline (speedup 1.0000x reference)
"""Trainium2 Bass kernel for GQA attention layer (B=1, T=2048, HID=4096,
32 q-heads / 8 kv-heads, head_dim 128, RoPE, causal) sharded over 8 cores.

Sharding: tensor-parallel over heads. Core c owns q-heads 4c..4c+3 and
kv-head c. Attention outputs (transposed, [512 hd, t]) are AllGathered;
each core then computes a 512-row slice of the output projection over the
full 4096 hd dims, so no AllReduce is needed. Host assembles the slices.

Matmuls run in bf16/fp16 (PE moving operand streams 2B/cycle),
accumulation in fp32 PSUM; softmax statistics in fp32.

Key performance structure (vs the naive pipeline):
- RoPE's partition swap uses DVE stream_shuffle. The head-dim rows of
  Wq/Wk (and cos/sin) are permuted on the host so each rotate pair sits
  within one 32-partition quadrant (scores contract over head dim, so a
  consistent permutation of q and k rows is a no-op mathematically).
- probs are fp16 with a constant exp offset (exp(s*scale - 9); the e^-9
  cancels in the softmax normalization). 16-bit probs enable 2x-rate DVE
  accumulation of the softmax denominator; one fp16 ones-matmul per
  (chunk, head) then does the cross-partition sum, replacing a full
  per-block denominator matmul stream (~60k PE cycles saved).
- Out-projections are interleaved mid-pipeline right after their gather
  completes; outproj(2) runs after attn(3) so the final half-gathers are
  fully hidden behind it.
- A warmup matmul burst during the initial DMA fill flips the PE HAM
  clock-gate to full rate before real work starts.
"""

import numpy as np

import concourse.bacc as bacc
import concourse.mybir as mybir
import concourse.tile as tile
from concourse.bass_utils import run_bass_kernel_spmd

T = 2048
HID = 4096
D = 128
N_HEADS = 32
N_KV = 8
HQ = N_HEADS // N_KV  # q heads per core (=4)
TT = 512  # t tile
NTT = T // TT  # 4
NH = HID // 128  # 32 h-tiles
SCALE = 1.0 / np.sqrt(np.float32(D))
EXP_OFF = -9.0  # exp offset; cancels in normalization, keeps probs in fp16
ROPE_BASE = 10000.0
N_CORES = 8

_F32 = mybir.dt.float32
_F16 = mybir.dt.float16
_DT = mybir.dt.bfloat16

# stream_shuffle swaps within each 32-partition quadrant: out[i] = in[i^16]
_SWAP_MASK = [i ^ 16 for i in range(32)]

_cached = None


def _build():
    nc = bacc.Bacc("TRN2", target_bir_lowering=False, debug=False, num_devices=N_CORES)

    xT = nc.dram_tensor("xT", [HID, T], _DT, kind="ExternalInput").ap()
    wqkvT = nc.dram_tensor(
        "wqkvT", [HID, (HQ + 2) * D], _DT, kind="ExternalInput"
    ).ap()
    woT = nc.dram_tensor("woT", [HID, HQ * D], _DT, kind="ExternalInput").ap()
    cos2 = nc.dram_tensor("cos2", [128, T], _DT, kind="ExternalInput").ap()
    sinS = nc.dram_tensor("sinS", [128, T], _DT, kind="ExternalInput").ap()
    masks = nc.dram_tensor("masks", [128, 128], _F16, kind="ExternalInput").ap()
    ones_i = nc.dram_tensor("ones_i", [128, 128], _F16, kind="ExternalInput").ap()
    ident_i = nc.dram_tensor("ident_i", [128, 128], _F16, kind="ExternalInput").ap()
    out = nc.dram_tensor("out", [HQ * D, T], _F32, kind="ExternalOutput").ap()

    Exp = mybir.ActivationFunctionType.Exp

    with tile.TileContext(nc) as tc:
        with (
            tc.tile_pool(name="const", bufs=1) as const,
            tc.tile_pool(name="big", bufs=1) as big,
            tc.tile_pool(name="sb", bufs=1) as sb,
            tc.tile_pool(name="ps", bufs=1, space="PSUM") as ps,
            tc.tile_pool(name="dram", bufs=1, space="DRAM") as dram,
        ):
            # ---- constants / persistent weights in SBUF ----
            cos_sb = const.tile([128, T], _DT, name="cos_sb")
            sin_sb = const.tile([128, T], _DT, name="sin_sb")
            mask_sb = const.tile([128, 128], _F16, name="mask_sb")
            ones_sb = const.tile([128, 128], _F16, name="ones_sb")
            ident_sb = const.tile([128, 128], _F16, name="ident_sb")
            bias_sb = const.tile([128, 1], _F32, name="bias_sb")
            nc.gpsimd.memset(bias_sb[:], EXP_OFF)
            wqkv_t = [
                const.tile([128, (HQ + 2) * D], _DT, name=f"wqkv_t{j}")
                for j in range(NH)
            ]
            wo_sb = const.tile([128, NH * HQ * D], _DT, name="wo_sb")

            nc.gpsimd.dma_start(out=ident_sb[:], in_=ident_i[:])
            nc.gpsimd.dma_start(out=ones_sb[:], in_=ones_i[:])
            nc.gpsimd.dma_start(out=cos_sb[:], in_=cos2[:])
            nc.gpsimd.dma_start(out=sin_sb[:], in_=sinS[:])
            nc.gpsimd.dma_start(out=mask_sb[:], in_=masks[:])

            qrot = [big.tile([128, T], _DT, name=f"qrot{h}") for h in range(HQ)]
            krot = big.tile([128, T], _DT, name="krot")
            v_sb = big.tile([128, T], _F16, name="v_sb")  # V[s,d]: block k at cols 128k

            attn_local = [
                dram.tile([HQ * D, TT], _DT, name=f"attn_local{i}") for i in range(NTT)
            ]
            attn_full = [
                dram.tile(
                    [N_CORES * HQ * D, TT],
                    _DT,
                    addr_space="Shared",
                    name=f"attn_full{i}",
                )
                for i in range(NTT)
            ]

            # ---- PE warmup: flip the HAM clock gate during the DMA fill ----
            warm_ps = ps.tile([128, 128], _F32, tag="p6", name="warm", bufs=2)
            for _ in range(28):
                nc.tensor.matmul(
                    warm_ps[:], ident_sb[:], ident_sb[:], start=True, stop=True
                )

            def proj(ti):
                tsl = slice(TT * ti, TT * (ti + 1))
                q_ps = [
                    ps.tile([128, TT], _F32, tag=f"p{h}", name=f"q_ps{h}")
                    for h in range(HQ)
                ]
                k_ps = ps.tile([128, TT], _F32, tag="p4")
                vT_ps = ps.tile([128, TT], _F32, tag="p5")
                for hi in range(NH):
                    hsl = slice(128 * hi, 128 * (hi + 1))
                    if ti == 0:
                        # weight slice DMA interleaved with x so the first
                        # matmuls aren't queued behind the whole preload
                        nc.sync.dma_start(out=wqkv_t[hi][:], in_=wqkvT[hsl, :])
                    elif ti == 1:
                        nc.sync.dma_start(
                            out=wo_sb[:, 512 * hi : 512 * (hi + 1)], in_=woT[hsl, :]
                        )
                    xt = sb.tile([128, TT], _DT, tag="x", bufs=26)
                    nc.sync.dma_start(out=xt[:], in_=xT[hsl, tsl])
                    st, sp = hi == 0, hi == NH - 1
                    for h in range(HQ):
                        nc.tensor.matmul(
                            q_ps[h][:],
                            wqkv_t[hi][:, 128 * h : 128 * (h + 1)],
                            xt[:],
                            start=st,
                            stop=sp,
                        )
                    nc.tensor.matmul(
                        k_ps[:],
                        wqkv_t[hi][:, HQ * D : (HQ + 1) * D],
                        xt[:],
                        start=st,
                        stop=sp,
                    )
                    nc.tensor.matmul(
                        vT_ps[:],
                        wqkv_t[hi][:, (HQ + 1) * D : (HQ + 2) * D],
                        xt[:],
                        start=st,
                        stop=sp,
                    )

                # V first: transpose [d, s] -> [s, d] blocks (fp16 for
                # uniform dtypes in the fp16 attnV/den matmuls), so v_sb and
                # the p5 bank are ready for the next chunk's early blocks
                vT_sb = sb.tile([128, TT], _F16, tag="vTs", bufs=2)
                nc.vector.tensor_copy(vT_sb[:], vT_ps[:])
                for j in range(TT // 128):
                    vtr = ps.tile([128, 128], _F16, tag="p6", bufs=2)
                    nc.tensor.transpose(
                        vtr[:], vT_sb[:, 128 * j : 128 * (j + 1)], ident_sb[:]
                    )
                    k = (TT // 128) * ti + j
                    nc.vector.tensor_copy(v_sb[:, 128 * k : 128 * (k + 1)], vtr[:])

                # RoPE: qrot = q*cos2 + shuffle(q)*sinS, with the partition
                # swap done by DVE stream_shuffle (weights pre-permuted on the
                # host so swap pairs sit within 32-partition quadrants).
                # bf16 intermediates get 2x DVE rate on the t2/add ops.
                order = [0, HQ, 1, 2, 3]
                stage = {}

                def rope_front(h):
                    src = q_ps[h] if h < HQ else k_ps
                    qs = sb.tile([128, TT], _F32, tag="qs", bufs=3)
                    nc.vector.stream_shuffle(qs[:], src[:], _SWAP_MASK)
                    t1 = sb.tile([128, TT], _DT, tag="t1", bufs=3)
                    nc.vector.tensor_mul(t1[:], src[:], cos_sb[:, tsl])
                    t2 = sb.tile([128, TT], _DT, tag="t2", bufs=3)
                    nc.vector.tensor_mul(t2[:], qs[:], sin_sb[:, tsl])
                    stage[h] = (t1, t2)

                def rope_back(h):
                    t1, t2 = stage.pop(h)
                    dst = qrot[h][:, tsl] if h < HQ else krot[:, tsl]
                    return nc.vector.tensor_add(dst, t1[:], t2[:])

                rope_front(order[0])
                for i, h in enumerate(order):
                    if i + 1 < len(order):
                        rope_front(order[i + 1])
                    last_rope = rope_back(h)

                return last_rope

            def attn(ti, split_gather=False):
                nblk = (TT // 128) * (ti + 1)
                for h in range(HQ):
                    # attn_ps/den alternate p4/p5 per head; scores rotate
                    # p0-p3 + double-buffered p6 — so a new head's score
                    # pipeline never waits on the previous head's epilogue
                    attn_ps = ps.tile(
                        [128, TT], _F32, tag="p4" if h % 2 == 0 else "p5"
                    )
                    den_tag = "p5" if h % 2 == 0 else "p4"
                    sc_tags = ["p0", "p1", "p2", "p3", "p6", "p6"]
                    probs_t = {}
                    acc = sb.tile([128, TT], _F16, tag="dacc", bufs=2, name=f"acc{h}")

                    def lo_of(k):
                        diag = k - (TT // 128) * ti
                        return 128 * diag if diag > 0 else 0

                    def emit_sc(k):
                        # scoresT block + exp (ACT); causal sub-range only
                        lo = lo_of(k)
                        diag = k - (TT // 128) * ti
                        qsl = slice(TT * ti + lo, TT * (ti + 1))
                        tg = sc_tags[k % 6]
                        sc = ps.tile(
                            [128, TT],
                            _F32,
                            tag=tg,
                            name=f"sc{k}",
                            bufs=2 if tg == "p6" else 1,
                        )
                        nc.tensor.matmul(
                            sc[:, lo:TT],
                            krot[:, 128 * k : 128 * (k + 1)],
                            qrot[h][:, qsl],
                            start=True,
                            stop=True,
                        )
                        probs = sb.tile([128, TT], _F16, tag="probs", bufs=7)
                        nc.scalar.activation(
                            probs[:, lo:TT], sc[:, lo:TT], Exp,
                            bias=bias_sb[:], scale=SCALE,
                        )
                        if diag >= 0:
                            # only the 128 diagonal columns need masking; the
                            # rest of the causal range is all-ones
                            nc.vector.tensor_mul(
                                probs[:, lo : lo + 128],
                                probs[:, lo : lo + 128],
                                mask_sb[:],
                            )
                        probs_t[k] = probs

                    for j in range(min(6, nblk)):
                        emit_sc(j)
                    for k in range(nblk):
                        if k + 6 < nblk:
                            emit_sc(k + 6)
                        lo = lo_of(k)
                        st, sp = k == 0, k == nblk - 1
                        probs = probs_t.pop(k)
                        nc.tensor.matmul(
                            attn_ps[:, lo:TT],
                            v_sb[:, 128 * k : 128 * (k + 1)],
                            probs[:, lo:TT],
                            start=st,
                            stop=sp,
                        )
                        # denominator accumulation off the PE (DVE 2x fp16)
                        if k == 0:
                            nc.vector.tensor_copy(acc[:], probs[:])
                        else:
                            nc.vector.tensor_add(
                                acc[:, lo:TT], acc[:, lo:TT], probs[:, lo:TT]
                            )
                    # cross-partition sum of acc: one fp16 ones-matmul
                    den_ps = ps.tile([128, TT], _F32, tag=den_tag, name=f"den{h}")
                    nc.tensor.matmul(
                        den_ps[:], ones_sb[:], acc[:], start=True, stop=True
                    )
                    recip = sb.tile([128, TT], _F32, tag="recip", bufs=2)
                    nc.vector.reciprocal_approx_fast(recip[:], den_ps[:])
                    anorm = sb.tile([128, TT], _DT, tag="anorm", bufs=2)
                    nc.vector.tensor_mul(anorm[:], attn_ps[:], recip[:])
                    nc.gpsimd.dma_start(
                        out=attn_local[ti][128 * h : 128 * (h + 1), :], in_=anorm[:]
                    )
                    if split_gather and h == 1:
                        gather_half(ti, 0)

            def gather(ti):
                nc.gpsimd.collective_compute(
                    "AllGather",
                    mybir.AluOpType.bypass,
                    replica_groups=[list(range(N_CORES))],
                    ins=[attn_local[ti].opt()],
                    outs=[attn_full[ti].opt()],
                )

            # the last t-chunk is gathered in two half-gathers (heads 0-1,
            # then 2-3) so the final output projection can start earlier
            attn_half = [
                dram.tile(
                    [N_CORES * 2 * D, TT], _DT, addr_space="Shared", name=f"attn_h{i}"
                )
                for i in range(2)
            ]

            def gather_half(ti, half):
                nc.gpsimd.collective_compute(
                    "AllGather",
                    mybir.AluOpType.bypass,
                    replica_groups=[list(range(N_CORES))],
                    ins=[attn_local[ti][256 * half : 256 * (half + 1), :]],
                    outs=[attn_half[half].opt()],
                )

            def outproj_pre(ti, n=6):
                # prefetch the first gathered-attention tiles during the
                # preceding attention chunk so outproj starts immediately
                tiles = []
                for hd in range(n):
                    ag = sb.tile([128, TT], _DT, tag="ag", bufs=8)
                    nc.sync.dma_start(
                        out=ag[:], in_=attn_full[ti][128 * hd : 128 * (hd + 1), :]
                    )
                    tiles.append(ag)
                return tiles

            def outproj(ti, pre=()):
                tags = ["p0", "p1", "p2", "p3"]
                o_ps = [
                    ps.tile([128, TT], _F32, tag=tg, name=f"o_ps{ti}_{i}")
                    for i, tg in enumerate(tags)
                ]
                for hd in range(NH):
                    if hd < len(pre):
                        ag = pre[hd]
                    else:
                        ag = sb.tile([128, TT], _DT, tag="ag", bufs=8)
                        nc.sync.dma_start(
                            out=ag[:], in_=attn_full[ti][128 * hd : 128 * (hd + 1), :]
                        )
                    st, sp = hd == 0, hd == NH - 1
                    for o in range(4):
                        nc.tensor.matmul(
                            o_ps[o][:],
                            wo_sb[:, 512 * hd + 128 * o : 512 * hd + 128 * (o + 1)],
                            ag[:],
                            start=st,
                            stop=sp,
                        )
                for o in range(4):
                    oc = sb.tile([128, TT], _F32, tag="oc", bufs=4)
                    nc.scalar.copy(oc[:], o_ps[o][:])
                    nc.gpsimd.dma_start(
                        out=out[128 * o : 128 * (o + 1), TT * ti : TT * (ti + 1)],
                        in_=oc[:],
                    )

            def outproj3():
                tags = ["p4", "p5", "p6", "p6"]
                o_ps = [
                    ps.tile(
                        [128, TT],
                        _F32,
                        tag=tg,
                        name=f"o_ps3_{i}",
                        bufs=2 if tg == "p6" else 1,
                    )
                    for i, tg in enumerate(tags)
                ]
                first = True
                for half in range(2):
                    for r in range(N_CORES):
                        for hp in range(2):
                            g = 4 * r + 2 * half + hp
                            row = 256 * r + 128 * hp
                            ag = sb.tile([128, TT], _DT, tag="ag", bufs=8)
                            nc.sync.dma_start(
                                out=ag[:], in_=attn_half[half][row : row + 128, :]
                            )
                            sp = half == 1 and r == N_CORES - 1 and hp == 1
                            for o in range(4):
                                nc.tensor.matmul(
                                    o_ps[o][:],
                                    wo_sb[
                                        :, 512 * g + 128 * o : 512 * g + 128 * (o + 1)
                                    ],
                                    ag[:],
                                    start=first,
                                    stop=sp,
                                )
                            first = False
                for o in range(4):
                    oc = sb.tile([128, TT], _F32, tag="oc", bufs=4)
                    nc.scalar.copy(oc[:], o_ps[o][:])
                    nc.gpsimd.dma_start(
                        out=out[128 * o : 128 * (o + 1), 3 * TT : 4 * TT], in_=oc[:]
                    )

            # pipeline: gathers issue right after their attention chunk and
            # run under the next projection; out-projections are interleaved;
            # outproj(2) runs after attn(3) so the final half-gathers finish
            # while it computes, leaving outproj3 with no collective wait
            proj(0)
            proj(1)
            attn(0)
            gather(0)
            attn(1)
            gather(1)
            proj(2)
            pre0 = outproj_pre(0)
            outproj(0, pre0)
            attn(2)
            gather(2)
            pre1 = outproj_pre(1)
            outproj(1, pre1)
            proj(3)
            pre2 = outproj_pre(2)
            attn(3, split_gather=True)
            gather_half(3, 1)
            outproj(2, pre2)
            outproj3()

    nc.compile()
    return nc


def _rope_perm():
    """Partition permutation: rope pair (i, i+64) lands 16 apart within one
    32-partition quadrant, so stream_shuffle(mask=i^16) does the swap."""
    r = np.zeros(128, dtype=np.int64)
    for p in range(128):
        q, j = divmod(p, 32)
        base = 16 * q
        r[p] = base + j if j < 16 else 64 + base + (j - 16)
    return r


def _host_inputs(hidden_states, Wq, Wk, Wv, Wo):
    import ml_dtypes

    bf16 = ml_dtypes.bfloat16
    x = np.asarray(hidden_states, dtype=np.float32).reshape(T, HID)
    xT = np.ascontiguousarray(x.T).astype(bf16)

    pos = np.arange(T, dtype=np.float32)
    inv_freq = ROPE_BASE ** (-np.arange(0, D, 2, dtype=np.float32) / D)  # [64]
    ang = pos[:, None] * inv_freq[None, :]  # [T, 64]
    cosT = np.cos(ang).T.astype(np.float32)  # [64, T]
    sinT = np.sin(ang).T.astype(np.float32)
    cos2 = np.concatenate([cosT, cosT], axis=0)
    sinS = np.concatenate([-sinT, sinT], axis=0)
    perm = _rope_perm()
    cos2 = np.ascontiguousarray(cos2[perm])
    sinS = np.ascontiguousarray(sinS[perm])

    p = np.arange(128)[:, None]
    tp = np.arange(128)[None, :]
    mask128 = np.ascontiguousarray((p <= tp).astype(np.float16))
    ones = np.ones((128, 128), dtype=np.float16)
    ident = np.eye(128, dtype=np.float16)

    Wq = np.asarray(Wq, dtype=np.float32)
    Wk = np.asarray(Wk, dtype=np.float32)
    Wv = np.asarray(Wv, dtype=np.float32)
    Wo = np.asarray(Wo, dtype=np.float32)

    in_maps = []
    for c in range(N_CORES):
        qs = slice(HQ * D * c, HQ * D * (c + 1))
        ks = slice(D * c, D * (c + 1))
        # permute the head-dim rows of each q head and of k (not v) so the
        # kernel's stream_shuffle rope swap is quadrant-local
        wq = Wq[qs, :].reshape(HQ, D, HID)[:, perm, :].reshape(HQ * D, HID)
        wk = Wk[ks, :][perm, :]
        in_maps.append(
            {
                "xT": xT,
                "wqkvT": np.ascontiguousarray(
                    np.concatenate([wq.T, wk.T, Wv[ks, :].T], axis=1)
                ).astype(bf16),
                "woT": np.ascontiguousarray(Wo[qs, :].T).astype(bf16),
                "cos2": cos2.astype(bf16),
                "sinS": sinS.astype(bf16),
                "masks": mask128,
                "ones_i": ones,
                "ident_i": ident,
            }
        )
    return in_maps


def get_program():
    global _cached
    if _cached is None:
        _cached = _build()
    return _cached


def kernel(hidden_states, Wq, Wk, Wv, Wo):
    nc = get_program()
    in_maps = _host_inputs(hidden_states, Wq, Wk, Wv, Wo)
    res = run_bass_kernel_spmd(nc, in_maps, list(range(N_CORES)))
    outT = np.concatenate([res.results[c]["out"] for c in range(N_CORES)], axis=0)
    return np.ascontiguousarray(outT.T).reshape(1, T, HID).astype(np.float32)


# revision 29
# speedup vs baseline: 1.0378x; 1.0378x over previous
"""Trainium2 Bass kernel for GQA attention layer (B=1, T=2048, HID=4096,
32 q-heads / 8 kv-heads, head_dim 128, RoPE, causal) sharded over 8 cores.

Sharding: tensor-parallel over heads. Core c owns q-heads 4c..4c+3 and
kv-head c. Attention outputs (transposed, [512 hd, t]) are AllGathered;
each core then computes a 512-row slice of the output projection over the
full 4096 hd dims, so no AllReduce is needed. Host assembles the slices.

Matmuls run in bf16/fp16 (PE moving operand streams 2B/cycle),
accumulation in fp32 PSUM; softmax statistics in fp32.

Key performance structure (vs the naive pipeline):
- RoPE's partition swap uses DVE stream_shuffle. The head-dim rows of
  Wq/Wk (and cos/sin) are permuted on the host so each rotate pair sits
  within one 32-partition quadrant (scores contract over head dim, so a
  consistent permutation of q and k rows is a no-op mathematically).
- probs are fp16 with a constant exp offset (exp(s*scale - 9); the e^-9
  cancels in the softmax normalization). 16-bit probs enable 2x-rate DVE
  accumulation of the softmax denominator; one fp16 ones-matmul per
  (chunk, head) then does the cross-partition sum, replacing a full
  per-block denominator matmul stream (~60k PE cycles saved).
- Out-projections are interleaved mid-pipeline right after their gather
  completes; outproj(2) runs after attn(3) so the final half-gathers are
  fully hidden behind it.
- A warmup matmul burst during the initial DMA fill flips the PE HAM
  clock-gate to full rate before real work starts.
"""

import numpy as np

import concourse.bacc as bacc
import concourse.mybir as mybir
import concourse.tile as tile
from concourse.bass_utils import run_bass_kernel_spmd

T = 2048
HID = 4096
D = 128
N_HEADS = 32
N_KV = 8
HQ = N_HEADS // N_KV  # q heads per core (=4)
TT = 512  # t tile
NTT = T // TT  # 4
NH = HID // 128  # 32 h-tiles
SCALE = 1.0 / np.sqrt(np.float32(D))
EXP_OFF = -9.0  # exp offset; cancels in normalization, keeps probs in fp16
ROPE_BASE = 10000.0
N_CORES = 8

_F32 = mybir.dt.float32
_F16 = mybir.dt.float16
_DT = mybir.dt.bfloat16

# stream_shuffle swaps within each 32-partition quadrant: out[i] = in[i^16]
_SWAP_MASK = [i ^ 16 for i in range(32)]

_cached = None


def _build():
    nc = bacc.Bacc("TRN2", target_bir_lowering=False, debug=False, num_devices=N_CORES)

    xT = nc.dram_tensor("xT", [HID, T], _DT, kind="ExternalInput").ap()
    wqkvT = nc.dram_tensor(
        "wqkvT", [HID, (HQ + 2) * D], _DT, kind="ExternalInput"
    ).ap()
    woT = nc.dram_tensor("woT", [HID, HQ * D], _DT, kind="ExternalInput").ap()
    cos2 = nc.dram_tensor("cos2", [128, T], _DT, kind="ExternalInput").ap()
    sinS = nc.dram_tensor("sinS", [128, T], _DT, kind="ExternalInput").ap()
    masks = nc.dram_tensor("masks", [128, 128], _F16, kind="ExternalInput").ap()
    ones_i = nc.dram_tensor("ones_i", [128, 128], _F16, kind="ExternalInput").ap()
    ident_i = nc.dram_tensor("ident_i", [128, 128], _F16, kind="ExternalInput").ap()
    out = nc.dram_tensor("out", [HQ * D, T], _F32, kind="ExternalOutput").ap()

    Exp = mybir.ActivationFunctionType.Exp

    with tile.TileContext(nc) as tc:
        with (
            tc.tile_pool(name="const", bufs=1) as const,
            tc.tile_pool(name="big", bufs=1) as big,
            tc.tile_pool(name="sb", bufs=1) as sb,
            tc.tile_pool(name="ps", bufs=1, space="PSUM") as ps,
            tc.tile_pool(name="dram", bufs=1, space="DRAM") as dram,
        ):
            # ---- constants / persistent weights in SBUF ----
            cos_sb = const.tile([128, T], _DT, name="cos_sb")
            sin_sb = const.tile([128, T], _DT, name="sin_sb")
            mask_sb = const.tile([128, 128], _F16, name="mask_sb")
            ones_sb = const.tile([128, 128], _F16, name="ones_sb")
            ident_sb = const.tile([128, 128], _F16, name="ident_sb")
            bias_sb = const.tile([128, 1], _F32, name="bias_sb")
            nc.gpsimd.memset(bias_sb[:], EXP_OFF)
            wqkv_t = [
                const.tile([128, (HQ + 2) * D], _DT, name=f"wqkv_t{j}")
                for j in range(NH)
            ]
            wo_sb = const.tile([128, NH * HQ * D], _DT, name="wo_sb")

            nc.gpsimd.dma_start(out=ident_sb[:], in_=ident_i[:])
            nc.gpsimd.dma_start(out=ones_sb[:], in_=ones_i[:])
            nc.gpsimd.dma_start(out=cos_sb[:], in_=cos2[:])
            nc.gpsimd.dma_start(out=sin_sb[:], in_=sinS[:])
            nc.gpsimd.dma_start(out=mask_sb[:], in_=masks[:])

            qrot = [big.tile([128, T], _DT, name=f"qrot{h}") for h in range(HQ)]
            krot = big.tile([128, T], _DT, name="krot")
            v_sb = big.tile([128, T], _F16, name="v_sb")  # V[s,d]: block k at cols 128k

            attn_local = [
                dram.tile([HQ * D, TT], _DT, name=f"attn_local{i}") for i in range(NTT)
            ]
            attn_full = [
                dram.tile(
                    [N_CORES * HQ * D, TT],
                    _DT,
                    addr_space="Shared",
                    name=f"attn_full{i}",
                )
                for i in range(NTT)
            ]

            # ---- PE warmup: flip the HAM clock gate during the DMA fill ----
            warm_ps = ps.tile([128, 128], _F32, tag="p6", name="warm", bufs=2)
            for _ in range(40):
                nc.tensor.matmul(
                    warm_ps[:], ident_sb[:], ident_sb[:], start=True, stop=True
                )

            def proj(ti):
                tsl = slice(TT * ti, TT * (ti + 1))
                q_ps = [
                    ps.tile([128, TT], _F32, tag=f"p{h}", name=f"q_ps{h}")
                    for h in range(HQ)
                ]
                k_ps = ps.tile([128, TT], _F32, tag="p4")
                vT_ps = ps.tile([128, TT], _F32, tag="p5")
                for hi in range(NH):
                    hsl = slice(128 * hi, 128 * (hi + 1))
                    if ti == 0:
                        # weight slice DMA interleaved with x so the first
                        # matmuls aren't queued behind the whole preload
                        nc.sync.dma_start(out=wqkv_t[hi][:], in_=wqkvT[hsl, :])
                    elif ti == 1:
                        nc.sync.dma_start(
                            out=wo_sb[:, 512 * hi : 512 * (hi + 1)], in_=woT[hsl, :]
                        )
                    xt = sb.tile([128, TT], _DT, tag="x", bufs=26)
                    nc.sync.dma_start(out=xt[:], in_=xT[hsl, tsl])
                    st, sp = hi == 0, hi == NH - 1
                    for h in range(HQ):
                        nc.tensor.matmul(
                            q_ps[h][:],
                            wqkv_t[hi][:, 128 * h : 128 * (h + 1)],
                            xt[:],
                            start=st,
                            stop=sp,
                        )
                    nc.tensor.matmul(
                        k_ps[:],
                        wqkv_t[hi][:, HQ * D : (HQ + 1) * D],
                        xt[:],
                        start=st,
                        stop=sp,
                    )
                    nc.tensor.matmul(
                        vT_ps[:],
                        wqkv_t[hi][:, (HQ + 1) * D : (HQ + 2) * D],
                        xt[:],
                        start=st,
                        stop=sp,
                    )

                # V first: transpose [d, s] -> [s, d] blocks (fp16 for
                # uniform dtypes in the fp16 attnV/den matmuls), so v_sb and
                # the p5 bank are ready for the next chunk's early blocks
                vT_sb = sb.tile([128, TT], _F16, tag="vTs", bufs=2)
                nc.vector.tensor_copy(vT_sb[:], vT_ps[:])
                for j in range(TT // 128):
                    vtr = ps.tile([128, 128], _F16, tag="p6", bufs=2)
                    nc.tensor.transpose(
                        vtr[:], vT_sb[:, 128 * j : 128 * (j + 1)], ident_sb[:]
                    )
                    k = (TT // 128) * ti + j
                    nc.vector.tensor_copy(v_sb[:, 128 * k : 128 * (k + 1)], vtr[:])

                # RoPE: qrot = q*cos2 + shuffle(q)*sinS, with the partition
                # swap done by DVE stream_shuffle (weights pre-permuted on the
                # host so swap pairs sit within 32-partition quadrants).
                # bf16 intermediates get 2x DVE rate on the t2/add ops.
                order = [0, HQ, 1, 2, 3]
                stage = {}

                def rope_front(h):
                    src = q_ps[h] if h < HQ else k_ps
                    qs = sb.tile([128, TT], _F32, tag="qs", bufs=3)
                    nc.vector.stream_shuffle(qs[:], src[:], _SWAP_MASK)
                    t1 = sb.tile([128, TT], _DT, tag="t1", bufs=3)
                    nc.vector.tensor_mul(t1[:], src[:], cos_sb[:, tsl])
                    t2 = sb.tile([128, TT], _DT, tag="t2", bufs=3)
                    nc.vector.tensor_mul(t2[:], qs[:], sin_sb[:, tsl])
                    stage[h] = (t1, t2)

                def rope_back(h):
                    t1, t2 = stage.pop(h)
                    dst = qrot[h][:, tsl] if h < HQ else krot[:, tsl]
                    return nc.vector.tensor_add(dst, t1[:], t2[:])

                rope_front(order[0])
                for i, h in enumerate(order):
                    if i + 1 < len(order):
                        rope_front(order[i + 1])
                    last_rope = rope_back(h)

                return last_rope

            def attn(ti, split_gather=False):
                nblk = (TT // 128) * (ti + 1)
                for h in range(HQ):
                    # attn_ps/den alternate p4/p5 per head; scores rotate
                    # p0-p3 + double-buffered p6 — so a new head's score
                    # pipeline never waits on the previous head's epilogue
                    attn_ps = ps.tile(
                        [128, TT], _F32, tag="p4" if h % 2 == 0 else "p5"
                    )
                    den_tag = "p5" if h % 2 == 0 else "p4"
                    sc_tags = ["p0", "p1", "p2", "p3", "p6", "p6"]
                    probs_t = {}
                    acc = sb.tile([128, TT], _F16, tag="dacc", bufs=3, name=f"acc{h}")

                    def lo_of(k):
                        diag = k - (TT // 128) * ti
                        return 128 * diag if diag > 0 else 0

                    def emit_sc(k):
                        # scoresT block + exp (ACT); causal sub-range only
                        lo = lo_of(k)
                        diag = k - (TT // 128) * ti
                        qsl = slice(TT * ti + lo, TT * (ti + 1))
                        tg = sc_tags[k % 6]
                        sc = ps.tile(
                            [128, TT],
                            _F32,
                            tag=tg,
                            name=f"sc{k}",
                            bufs=2 if tg == "p6" else 1,
                        )
                        nc.tensor.matmul(
                            sc[:, lo:TT],
                            krot[:, 128 * k : 128 * (k + 1)],
                            qrot[h][:, qsl],
                            start=True,
                            stop=True,
                        )
                        probs = sb.tile([128, TT], _F16, tag="probs", bufs=9)
                        nc.scalar.activation(
                            probs[:, lo:TT], sc[:, lo:TT], Exp,
                            bias=bias_sb[:], scale=SCALE,
                        )
                        if diag >= 0:
                            # only the 128 diagonal columns need masking; the
                            # rest of the causal range is all-ones
                            nc.vector.tensor_mul(
                                probs[:, lo : lo + 128],
                                probs[:, lo : lo + 128],
                                mask_sb[:],
                            )
                        probs_t[k] = probs

                    for j in range(min(6, nblk)):
                        emit_sc(j)
                    for k in range(nblk):
                        if k + 6 < nblk:
                            emit_sc(k + 6)
                        lo = lo_of(k)
                        st, sp = k == 0, k == nblk - 1
                        probs = probs_t.pop(k)
                        nc.tensor.matmul(
                            attn_ps[:, lo:TT],
                            v_sb[:, 128 * k : 128 * (k + 1)],
                            probs[:, lo:TT],
                            start=st,
                            stop=sp,
                        )
                        # denominator accumulation off the PE (DVE 2x fp16)
                        if k == 0:
                            nc.vector.tensor_copy(acc[:], probs[:])
                        else:
                            nc.vector.tensor_add(
                                acc[:, lo:TT], acc[:, lo:TT], probs[:, lo:TT]
                            )
                    # cross-partition sum of acc: one fp16 ones-matmul
                    den_ps = ps.tile([128, TT], _F32, tag=den_tag, name=f"den{h}")
                    nc.tensor.matmul(
                        den_ps[:], ones_sb[:], acc[:], start=True, stop=True
                    )
                    recip = sb.tile([128, TT], _F32, tag="recip", bufs=3)
                    nc.vector.reciprocal_approx_fast(recip[:], den_ps[:])
                    anorm = sb.tile([128, TT], _DT, tag="anorm", bufs=3)
                    nc.vector.tensor_mul(anorm[:], attn_ps[:], recip[:])
                    nc.gpsimd.dma_start(
                        out=attn_local[ti][128 * h : 128 * (h + 1), :], in_=anorm[:]
                    )
                    if split_gather and h == 1:
                        gather_half(ti, 0)

            def gather(ti):
                nc.gpsimd.collective_compute(
                    "AllGather",
                    mybir.AluOpType.bypass,
                    replica_groups=[list(range(N_CORES))],
                    ins=[attn_local[ti].opt()],
                    outs=[attn_full[ti].opt()],
                )

            # the last t-chunk is gathered in two half-gathers (heads 0-1,
            # then 2-3) so the final output projection can start earlier
            attn_half = [
                dram.tile(
                    [N_CORES * 2 * D, TT], _DT, addr_space="Shared", name=f"attn_h{i}"
                )
                for i in range(2)
            ]

            def gather_half(ti, half):
                nc.gpsimd.collective_compute(
                    "AllGather",
                    mybir.AluOpType.bypass,
                    replica_groups=[list(range(N_CORES))],
                    ins=[attn_local[ti][256 * half : 256 * (half + 1), :]],
                    outs=[attn_half[half].opt()],
                )

            def outproj_pre(ti, n=6):
                # prefetch the first gathered-attention tiles during the
                # preceding attention chunk so outproj starts immediately
                tiles = []
                for hd in range(n):
                    ag = sb.tile([128, TT], _DT, tag="ag", bufs=8)
                    nc.sync.dma_start(
                        out=ag[:], in_=attn_full[ti][128 * hd : 128 * (hd + 1), :]
                    )
                    tiles.append(ag)
                return tiles

            def outproj(ti, pre=()):
                tags = ["p0", "p1", "p2", "p3"]
                o_ps = [
                    ps.tile([128, TT], _F32, tag=tg, name=f"o_ps{ti}_{i}")
                    for i, tg in enumerate(tags)
                ]
                for hd in range(NH):
                    if hd < len(pre):
                        ag = pre[hd]
                    else:
                        ag = sb.tile([128, TT], _DT, tag="ag", bufs=8)
                        nc.sync.dma_start(
                            out=ag[:], in_=attn_full[ti][128 * hd : 128 * (hd + 1), :]
                        )
                    st, sp = hd == 0, hd == NH - 1
                    for o in range(4):
                        nc.tensor.matmul(
                            o_ps[o][:],
                            wo_sb[:, 512 * hd + 128 * o : 512 * hd + 128 * (o + 1)],
                            ag[:],
                            start=st,
                            stop=sp,
                        )
                for o in range(4):
                    oc = sb.tile([128, TT], _F32, tag="oc", bufs=4)
                    nc.scalar.copy(oc[:], o_ps[o][:])
                    nc.gpsimd.dma_start(
                        out=out[128 * o : 128 * (o + 1), TT * ti : TT * (ti + 1)],
                        in_=oc[:],
                    )

            def outproj3():
                tags = ["p4", "p5", "p6", "p6"]
                o_ps = [
                    ps.tile(
                        [128, TT],
                        _F32,
                        tag=tg,
                        name=f"o_ps3_{i}",
                        bufs=2 if tg == "p6" else 1,
                    )
                    for i, tg in enumerate(tags)
                ]
                first = True
                for half in range(2):
                    for r in range(N_CORES):
                        for hp in range(2):
                            g = 4 * r + 2 * half + hp
                            row = 256 * r + 128 * hp
                            ag = sb.tile([128, TT], _DT, tag="ag", bufs=8)
                            nc.sync.dma_start(
                                out=ag[:], in_=attn_half[half][row : row + 128, :]
                            )
                            sp = half == 1 and r == N_CORES - 1 and hp == 1
                            for o in range(4):
                                nc.tensor.matmul(
                                    o_ps[o][:],
                                    wo_sb[
                                        :, 512 * g + 128 * o : 512 * g + 128 * (o + 1)
                                    ],
                                    ag[:],
                                    start=first,
                                    stop=sp,
                                )
                            first = False
                for o in range(4):
                    oc = sb.tile([128, TT], _F32, tag="oc", bufs=4)
                    nc.scalar.copy(oc[:], o_ps[o][:])
                    nc.gpsimd.dma_start(
                        out=out[128 * o : 128 * (o + 1), 3 * TT : 4 * TT], in_=oc[:]
                    )

            # pipeline: gathers issue right after their attention chunk and
            # run under the next projection; out-projections are interleaved;
            # outproj(2) runs after attn(3) so the final half-gathers finish
            # while it computes, leaving outproj3 with no collective wait
            proj(0)
            attn(0)
            gather(0)
            proj(1)
            pre0 = outproj_pre(0)
            attn(1)
            gather(1)
            outproj(0, pre0)
            proj(2)
            pre1 = outproj_pre(1)
            attn(2)
            gather(2)
            outproj(1, pre1)
            proj(3)
            pre2 = outproj_pre(2)
            attn(3, split_gather=True)
            gather_half(3, 1)
            outproj(2, pre2)
            outproj3()

    nc.compile()
    return nc


def _rope_perm():
    """Partition permutation: rope pair (i, i+64) lands 16 apart within one
    32-partition quadrant, so stream_shuffle(mask=i^16) does the swap."""
    r = np.zeros(128, dtype=np.int64)
    for p in range(128):
        q, j = divmod(p, 32)
        base = 16 * q
        r[p] = base + j if j < 16 else 64 + base + (j - 16)
    return r


def _host_inputs(hidden_states, Wq, Wk, Wv, Wo):
    import ml_dtypes

    bf16 = ml_dtypes.bfloat16
    x = np.asarray(hidden_states, dtype=np.float32).reshape(T, HID)
    xT = np.ascontiguousarray(x.T).astype(bf16)

    pos = np.arange(T, dtype=np.float32)
    inv_freq = ROPE_BASE ** (-np.arange(0, D, 2, dtype=np.float32) / D)  # [64]
    ang = pos[:, None] * inv_freq[None, :]  # [T, 64]
    cosT = np.cos(ang).T.astype(np.float32)  # [64, T]
    sinT = np.sin(ang).T.astype(np.float32)
    cos2 = np.concatenate([cosT, cosT], axis=0)
    sinS = np.concatenate([-sinT, sinT], axis=0)
    perm = _rope_perm()
    cos2 = np.ascontiguousarray(cos2[perm])
    sinS = np.ascontiguousarray(sinS[perm])

    p = np.arange(128)[:, None]
    tp = np.arange(128)[None, :]
    mask128 = np.ascontiguousarray((p <= tp).astype(np.float16))
    ones = np.ones((128, 128), dtype=np.float16)
    ident = np.eye(128, dtype=np.float16)

    Wq = np.asarray(Wq, dtype=np.float32)
    Wk = np.asarray(Wk, dtype=np.float32)
    Wv = np.asarray(Wv, dtype=np.float32)
    Wo = np.asarray(Wo, dtype=np.float32)

    in_maps = []
    for c in range(N_CORES):
        qs = slice(HQ * D * c, HQ * D * (c + 1))
        ks = slice(D * c, D * (c + 1))
        # permute the head-dim rows of each q head and of k (not v) so the
        # kernel's stream_shuffle rope swap is quadrant-local
        wq = Wq[qs, :].reshape(HQ, D, HID)[:, perm, :].reshape(HQ * D, HID)
        wk = Wk[ks, :][perm, :]
        in_maps.append(
            {
                "xT": xT,
                "wqkvT": np.ascontiguousarray(
                    np.concatenate([wq.T, wk.T, Wv[ks, :].T], axis=1)
                ).astype(bf16),
                "woT": np.ascontiguousarray(Wo[qs, :].T).astype(bf16),
                "cos2": cos2.astype(bf16),
                "sinS": sinS.astype(bf16),
                "masks": mask128,
                "ones_i": ones,
                "ident_i": ident,
            }
        )
    return in_maps


def get_program():
    global _cached
    if _cached is None:
        _cached = _build()
    return _cached


def kernel(hidden_states, Wq, Wk, Wv, Wo):
    nc = get_program()
    in_maps = _host_inputs(hidden_states, Wq, Wk, Wv, Wo)
    res = run_bass_kernel_spmd(nc, in_maps, list(range(N_CORES)))
    outT = np.concatenate([res.results[c]["out"] for c in range(N_CORES)], axis=0)
    return np.ascontiguousarray(outT.T).reshape(1, T, HID).astype(np.float32)


# revision 31
# speedup vs baseline: 1.0653x; 1.0266x over previous
"""Trainium2 Bass kernel for GQA attention layer (B=1, T=2048, HID=4096,
32 q-heads / 8 kv-heads, head_dim 128, RoPE, causal) sharded over 8 cores.

Sharding: tensor-parallel over heads. Core c owns q-heads 4c..4c+3 and
kv-head c. Attention outputs (transposed, [512 hd, t]) are AllGathered;
each core then computes a 512-row slice of the output projection over the
full 4096 hd dims, so no AllReduce is needed. Host assembles the slices.

Matmuls run in bf16/fp16 (PE moving operand streams 2B/cycle),
accumulation in fp32 PSUM; softmax statistics in fp32.

Key performance structure (vs the naive pipeline):
- RoPE's partition swap uses DVE stream_shuffle. The head-dim rows of
  Wq/Wk (and cos/sin) are permuted on the host so each rotate pair sits
  within one 32-partition quadrant (scores contract over head dim, so a
  consistent permutation of q and k rows is a no-op mathematically).
- probs are fp16 with a constant exp offset (exp(s*scale - 9); the e^-9
  cancels in the softmax normalization). 16-bit probs enable 2x-rate DVE
  accumulation of the softmax denominator; one fp16 ones-matmul per
  (chunk, head) then does the cross-partition sum, replacing a full
  per-block denominator matmul stream (~60k PE cycles saved).
- Out-projections are interleaved mid-pipeline right after their gather
  completes; outproj(2) runs after attn(3) so the final half-gathers are
  fully hidden behind it.
- A warmup matmul burst during the initial DMA fill flips the PE HAM
  clock-gate to full rate before real work starts.
"""

import numpy as np

import concourse.bacc as bacc
import concourse.mybir as mybir
import concourse.tile as tile
from concourse.bass_utils import run_bass_kernel_spmd

T = 2048
HID = 4096
D = 128
N_HEADS = 32
N_KV = 8
HQ = N_HEADS // N_KV  # q heads per core (=4)
TT = 512  # t tile
NTT = T // TT  # 4
NH = HID // 128  # 32 h-tiles
SCALE = 1.0 / np.sqrt(np.float32(D))
EXP_OFF = -9.0  # exp offset; cancels in normalization, keeps probs in fp16
ROPE_BASE = 10000.0
N_CORES = 8

_F32 = mybir.dt.float32
_F16 = mybir.dt.float16
_DT = mybir.dt.bfloat16

# stream_shuffle swaps within each 32-partition quadrant: out[i] = in[i^16]
_SWAP_MASK = [i ^ 16 for i in range(32)]

_cached = None


def _build():
    nc = bacc.Bacc("TRN2", target_bir_lowering=False, debug=False, num_devices=N_CORES)

    xT = nc.dram_tensor("xT", [HID, T], _DT, kind="ExternalInput").ap()
    wqkvT = nc.dram_tensor(
        "wqkvT", [HID, (HQ + 2) * D], _DT, kind="ExternalInput"
    ).ap()
    woT = nc.dram_tensor("woT", [HID, HQ * D], _DT, kind="ExternalInput").ap()
    cos2 = nc.dram_tensor("cos2", [128, T], _DT, kind="ExternalInput").ap()
    sinS = nc.dram_tensor("sinS", [128, T], _DT, kind="ExternalInput").ap()
    masks = nc.dram_tensor("masks", [128, 128], _F16, kind="ExternalInput").ap()
    ones_i = nc.dram_tensor("ones_i", [128, 128], _F16, kind="ExternalInput").ap()
    ident_i = nc.dram_tensor("ident_i", [128, 128], _F16, kind="ExternalInput").ap()
    out = nc.dram_tensor("out", [HQ * D, T], _F32, kind="ExternalOutput").ap()

    Exp = mybir.ActivationFunctionType.Exp

    with tile.TileContext(nc) as tc:
        with (
            tc.tile_pool(name="const", bufs=1) as const,
            tc.tile_pool(name="big", bufs=1) as big,
            tc.tile_pool(name="sb", bufs=1) as sb,
            tc.tile_pool(name="ps", bufs=1, space="PSUM") as ps,
            tc.tile_pool(name="dram", bufs=1, space="DRAM") as dram,
        ):
            # ---- constants / persistent weights in SBUF ----
            cos_sb = const.tile([128, T], _DT, name="cos_sb")
            sin_sb = const.tile([128, T], _DT, name="sin_sb")
            mask_sb = const.tile([128, 128], _F16, name="mask_sb")
            ones_sb = const.tile([128, 128], _F16, name="ones_sb")
            ident_sb = const.tile([128, 128], _F16, name="ident_sb")
            bias_sb = const.tile([128, 1], _F32, name="bias_sb")
            nc.gpsimd.memset(bias_sb[:], EXP_OFF)
            wqkv_t = [
                const.tile([128, (HQ + 2) * D], _DT, name=f"wqkv_t{j}")
                for j in range(NH)
            ]
            wo_sb = const.tile([128, NH * HQ * D], _DT, name="wo_sb")

            nc.gpsimd.dma_start(out=ident_sb[:], in_=ident_i[:])
            nc.gpsimd.dma_start(out=ones_sb[:], in_=ones_i[:])
            nc.gpsimd.dma_start(out=cos_sb[:], in_=cos2[:])
            nc.gpsimd.dma_start(out=sin_sb[:], in_=sinS[:])
            nc.gpsimd.dma_start(out=mask_sb[:], in_=masks[:])

            qrot = [big.tile([128, T], _DT, name=f"qrot{h}") for h in range(HQ)]
            krot = big.tile([128, T], _DT, name="krot")
            v_sb = big.tile([128, T], _F16, name="v_sb")  # V[s,d]: block k at cols 128k

            attn_local = [
                dram.tile([HQ * D, TT], _DT, name=f"attn_local{i}") for i in range(NTT)
            ]
            attn_full = [
                dram.tile(
                    [N_CORES * HQ * D, TT],
                    _DT,
                    addr_space="Shared",
                    name=f"attn_full{i}",
                )
                for i in range(NTT)
            ]

            # ---- PE warmup: flip the HAM clock gate during the DMA fill ----
            warm_ps = ps.tile([128, 128], _F32, tag="p6", name="warm", bufs=2)
            for _ in range(28):
                nc.tensor.matmul(
                    warm_ps[:], ident_sb[:], ident_sb[:], start=True, stop=True
                )

            def proj(ti):
                tsl = slice(TT * ti, TT * (ti + 1))
                q_ps = [
                    ps.tile([128, TT], _F32, tag=f"p{h}", name=f"q_ps{h}")
                    for h in range(HQ)
                ]
                k_ps = ps.tile([128, TT], _F32, tag="p4")
                vT_ps = ps.tile([128, TT], _F32, tag="p5")

                # RoPE: qrot = q*cos2 + shuffle(q)*sinS, with the partition
                # swap done by DVE stream_shuffle (weights pre-permuted on the
                # host so swap pairs sit within 32-partition quadrants).
                # bf16 intermediates get 2x DVE rate on the t2/add ops.
                stage = {}

                def rope_front(h):
                    src = q_ps[h] if h < HQ else k_ps
                    qs = sb.tile([128, TT], _F32, tag="qs", bufs=3)
                    nc.vector.stream_shuffle(qs[:], src[:], _SWAP_MASK)
                    t1 = sb.tile([128, TT], _DT, tag="t1", bufs=3)
                    nc.vector.tensor_mul(t1[:], src[:], cos_sb[:, tsl])
                    t2 = sb.tile([128, TT], _DT, tag="t2", bufs=3)
                    nc.vector.tensor_mul(t2[:], qs[:], sin_sb[:, tsl])
                    stage[h] = (t1, t2)

                def rope_back(h):
                    t1, t2 = stage.pop(h)
                    dst = qrot[h][:, tsl] if h < HQ else krot[:, tsl]
                    return nc.vector.tensor_add(dst, t1[:], t2[:])

                def vtrans():
                    # V: transpose [d, s] -> [s, d] blocks (fp16 for uniform
                    # dtypes in the fp16 attnV/den matmuls)
                    vT_sb = sb.tile([128, TT], _F16, tag="vTs", bufs=2)
                    nc.vector.tensor_copy(vT_sb[:], vT_ps[:])
                    for j in range(TT // 128):
                        vtr = ps.tile([128, 128], _F16, tag="p6", bufs=2)
                        nc.tensor.transpose(
                            vtr[:], vT_sb[:, 128 * j : 128 * (j + 1)], ident_sb[:]
                        )
                        k = (TT // 128) * ti + j
                        nc.vector.tensor_copy(v_sb[:, 128 * k : 128 * (k + 1)], vtr[:])

                if ti == 0:
                    # first chunk: x streams in live, so interleave all six
                    # matmul streams per h-tile (DMA-paced anyway)
                    for hi in range(NH):
                        hsl = slice(128 * hi, 128 * (hi + 1))
                        # weight slice DMA interleaved with x so the first
                        # matmuls aren't queued behind the whole preload
                        nc.sync.dma_start(out=wqkv_t[hi][:], in_=wqkvT[hsl, :])
                        xt = sb.tile([128, TT], _DT, tag="x", bufs=34)
                        nc.sync.dma_start(out=xt[:], in_=xT[hsl, tsl])
                        st, sp = hi == 0, hi == NH - 1
                        for h in range(HQ):
                            nc.tensor.matmul(
                                q_ps[h][:],
                                wqkv_t[hi][:, 128 * h : 128 * (h + 1)],
                                xt[:],
                                start=st,
                                stop=sp,
                            )
                        nc.tensor.matmul(
                            k_ps[:],
                            wqkv_t[hi][:, HQ * D : (HQ + 1) * D],
                            xt[:],
                            start=st,
                            stop=sp,
                        )
                        nc.tensor.matmul(
                            vT_ps[:],
                            wqkv_t[hi][:, (HQ + 1) * D : (HQ + 2) * D],
                            xt[:],
                            start=st,
                            stop=sp,
                        )
                    vtrans()
                    order = [0, HQ, 1, 2, 3]
                    rope_front(order[0])
                    for i, h in enumerate(order):
                        if i + 1 < len(order):
                            rope_front(order[i + 1])
                        last_rope = rope_back(h)
                    return last_rope

                # chunks 1-3: x tiles were prefetched during the previous
                # attention window, so run the q0 and k streams as their own
                # early passes — their RoPE chains then complete ~30us before
                # proj ends, removing the proj->attn transition stall
                xts = []
                for hi in range(NH):
                    hsl = slice(128 * hi, 128 * (hi + 1))
                    if ti == 1:
                        nc.sync.dma_start(
                            out=wo_sb[:, 512 * hi : 512 * (hi + 1)], in_=woT[hsl, :]
                        )
                    xt = sb.tile([128, TT], _DT, tag="x", bufs=34)
                    nc.sync.dma_start(out=xt[:], in_=xT[hsl, tsl])
                    xts.append(xt)
                for hi in range(NH):
                    nc.tensor.matmul(
                        q_ps[0][:],
                        wqkv_t[hi][:, 0:128],
                        xts[hi][:],
                        start=hi == 0,
                        stop=hi == NH - 1,
                    )
                rope_front(0)
                for hi in range(NH):
                    nc.tensor.matmul(
                        k_ps[:],
                        wqkv_t[hi][:, HQ * D : (HQ + 1) * D],
                        xts[hi][:],
                        start=hi == 0,
                        stop=hi == NH - 1,
                    )
                rope_front(HQ)
                rope_back(0)
                for hi in range(NH):
                    st, sp = hi == 0, hi == NH - 1
                    for h in range(1, HQ):
                        nc.tensor.matmul(
                            q_ps[h][:],
                            wqkv_t[hi][:, 128 * h : 128 * (h + 1)],
                            xts[hi][:],
                            start=st,
                            stop=sp,
                        )
                    nc.tensor.matmul(
                        vT_ps[:],
                        wqkv_t[hi][:, (HQ + 1) * D : (HQ + 2) * D],
                        xts[hi][:],
                        start=st,
                        stop=sp,
                    )
                rope_back(HQ)
                vtrans()
                order = [1, 2, 3]
                rope_front(order[0])
                for i, h in enumerate(order):
                    if i + 1 < len(order):
                        rope_front(order[i + 1])
                    last_rope = rope_back(h)
                return last_rope

            def attn(ti, split_gather=False):
                nblk = (TT // 128) * (ti + 1)
                for h in range(HQ):
                    # attn_ps/den alternate p4/p5 per head; scores rotate
                    # p0-p3 + double-buffered p6 — so a new head's score
                    # pipeline never waits on the previous head's epilogue
                    attn_ps = ps.tile(
                        [128, TT], _F32, tag="p4" if h % 2 == 0 else "p5"
                    )
                    den_tag = "p5" if h % 2 == 0 else "p4"
                    sc_tags = ["p0", "p1", "p2", "p3", "p6", "p6"]
                    probs_t = {}
                    acc = sb.tile([128, TT], _F16, tag="dacc", bufs=2, name=f"acc{h}")

                    def lo_of(k):
                        diag = k - (TT // 128) * ti
                        return 128 * diag if diag > 0 else 0

                    def emit_sc(k):
                        # scoresT block + exp (ACT); causal sub-range only
                        lo = lo_of(k)
                        diag = k - (TT // 128) * ti
                        qsl = slice(TT * ti + lo, TT * (ti + 1))
                        tg = sc_tags[k % 6]
                        sc = ps.tile(
                            [128, TT],
                            _F32,
                            tag=tg,
                            name=f"sc{k}",
                            bufs=2 if tg == "p6" else 1,
                        )
                        nc.tensor.matmul(
                            sc[:, lo:TT],
                            krot[:, 128 * k : 128 * (k + 1)],
                            qrot[h][:, qsl],
                            start=True,
                            stop=True,
                        )
                        probs = sb.tile([128, TT], _F16, tag="probs", bufs=7)
                        nc.scalar.activation(
                            probs[:, lo:TT], sc[:, lo:TT], Exp,
                            bias=bias_sb[:], scale=SCALE,
                        )
                        if diag >= 0:
                            # only the 128 diagonal columns need masking; the
                            # rest of the causal range is all-ones
                            nc.vector.tensor_mul(
                                probs[:, lo : lo + 128],
                                probs[:, lo : lo + 128],
                                mask_sb[:],
                            )
                        probs_t[k] = probs

                    for j in range(min(6, nblk)):
                        emit_sc(j)
                    for k in range(nblk):
                        if k + 6 < nblk:
                            emit_sc(k + 6)
                        lo = lo_of(k)
                        st, sp = k == 0, k == nblk - 1
                        probs = probs_t.pop(k)
                        nc.tensor.matmul(
                            attn_ps[:, lo:TT],
                            v_sb[:, 128 * k : 128 * (k + 1)],
                            probs[:, lo:TT],
                            start=st,
                            stop=sp,
                        )
                        # denominator accumulation off the PE (DVE 2x fp16)
                        if k == 0:
                            nc.vector.tensor_copy(acc[:], probs[:])
                        else:
                            nc.vector.tensor_add(
                                acc[:, lo:TT], acc[:, lo:TT], probs[:, lo:TT]
                            )
                    # cross-partition sum of acc: one fp16 ones-matmul
                    den_ps = ps.tile([128, TT], _F32, tag=den_tag, name=f"den{h}")
                    nc.tensor.matmul(
                        den_ps[:], ones_sb[:], acc[:], start=True, stop=True
                    )
                    recip = sb.tile([128, TT], _F32, tag="recip", bufs=2)
                    nc.vector.reciprocal_approx_fast(recip[:], den_ps[:])
                    anorm = sb.tile([128, TT], _DT, tag="anorm", bufs=2)
                    nc.vector.tensor_mul(anorm[:], attn_ps[:], recip[:])
                    nc.gpsimd.dma_start(
                        out=attn_local[ti][128 * h : 128 * (h + 1), :], in_=anorm[:]
                    )
                    if split_gather and h == 1:
                        gather_half(ti, 0)

            def gather(ti):
                nc.gpsimd.collective_compute(
                    "AllGather",
                    mybir.AluOpType.bypass,
                    replica_groups=[list(range(N_CORES))],
                    ins=[attn_local[ti].opt()],
                    outs=[attn_full[ti].opt()],
                )

            # the last t-chunk is gathered in two half-gathers (heads 0-1,
            # then 2-3) so the final output projection can start earlier
            attn_half = [
                dram.tile(
                    [N_CORES * 2 * D, TT], _DT, addr_space="Shared", name=f"attn_h{i}"
                )
                for i in range(2)
            ]

            def gather_half(ti, half):
                nc.gpsimd.collective_compute(
                    "AllGather",
                    mybir.AluOpType.bypass,
                    replica_groups=[list(range(N_CORES))],
                    ins=[attn_local[ti][256 * half : 256 * (half + 1), :]],
                    outs=[attn_half[half].opt()],
                )

            def outproj_pre(ti, n=6):
                # prefetch the first gathered-attention tiles during the
                # preceding attention chunk so outproj starts immediately
                tiles = []
                for hd in range(n):
                    ag = sb.tile([128, TT], _DT, tag="ag", bufs=8)
                    nc.sync.dma_start(
                        out=ag[:], in_=attn_full[ti][128 * hd : 128 * (hd + 1), :]
                    )
                    tiles.append(ag)
                return tiles

            def outproj(ti, pre=()):
                tags = ["p0", "p1", "p2", "p3"]
                o_ps = [
                    ps.tile([128, TT], _F32, tag=tg, name=f"o_ps{ti}_{i}")
                    for i, tg in enumerate(tags)
                ]
                for hd in range(NH):
                    if hd < len(pre):
                        ag = pre[hd]
                    else:
                        ag = sb.tile([128, TT], _DT, tag="ag", bufs=8)
                        nc.sync.dma_start(
                            out=ag[:], in_=attn_full[ti][128 * hd : 128 * (hd + 1), :]
                        )
                    st, sp = hd == 0, hd == NH - 1
                    for o in range(4):
                        nc.tensor.matmul(
                            o_ps[o][:],
                            wo_sb[:, 512 * hd + 128 * o : 512 * hd + 128 * (o + 1)],
                            ag[:],
                            start=st,
                            stop=sp,
                        )
                for o in range(4):
                    oc = sb.tile([128, TT], _F32, tag="oc", bufs=4)
                    nc.scalar.copy(oc[:], o_ps[o][:])
                    nc.gpsimd.dma_start(
                        out=out[128 * o : 128 * (o + 1), TT * ti : TT * (ti + 1)],
                        in_=oc[:],
                    )

            def outproj3():
                tags = ["p4", "p5", "p6", "p6"]
                o_ps = [
                    ps.tile(
                        [128, TT],
                        _F32,
                        tag=tg,
                        name=f"o_ps3_{i}",
                        bufs=2 if tg == "p6" else 1,
                    )
                    for i, tg in enumerate(tags)
                ]
                first = True
                for half in range(2):
                    for r in range(N_CORES):
                        for hp in range(2):
                            g = 4 * r + 2 * half + hp
                            row = 256 * r + 128 * hp
                            ag = sb.tile([128, TT], _DT, tag="ag", bufs=8)
                            nc.sync.dma_start(
                                out=ag[:], in_=attn_half[half][row : row + 128, :]
                            )
                            sp = half == 1 and r == N_CORES - 1 and hp == 1
                            for o in range(4):
                                nc.tensor.matmul(
                                    o_ps[o][:],
                                    wo_sb[
                                        :, 512 * g + 128 * o : 512 * g + 128 * (o + 1)
                                    ],
                                    ag[:],
                                    start=first,
                                    stop=sp,
                                )
                            first = False
                for o in range(4):
                    oc = sb.tile([128, TT], _F32, tag="oc", bufs=4)
                    nc.scalar.copy(oc[:], o_ps[o][:])
                    nc.gpsimd.dma_start(
                        out=out[128 * o : 128 * (o + 1), 3 * TT : 4 * TT], in_=oc[:]
                    )

            # pipeline: gathers issue right after their attention chunk and
            # run under the next projection; out-projections are interleaved;
            # outproj(2) runs after attn(3) so the final half-gathers finish
            # while it computes, leaving outproj3 with no collective wait
            proj(0)
            attn(0)
            gather(0)
            proj(1)
            pre0 = outproj_pre(0)
            attn(1)
            gather(1)
            outproj(0, pre0)
            proj(2)
            pre1 = outproj_pre(1)
            attn(2)
            gather(2)
            outproj(1, pre1)
            proj(3)
            pre2 = outproj_pre(2)
            attn(3, split_gather=True)
            gather_half(3, 1)
            outproj(2, pre2)
            outproj3()

    nc.compile()
    return nc


def _rope_perm():
    """Partition permutation: rope pair (i, i+64) lands 16 apart within one
    32-partition quadrant, so stream_shuffle(mask=i^16) does the swap."""
    r = np.zeros(128, dtype=np.int64)
    for p in range(128):
        q, j = divmod(p, 32)
        base = 16 * q
        r[p] = base + j if j < 16 else 64 + base + (j - 16)
    return r


def _host_inputs(hidden_states, Wq, Wk, Wv, Wo):
    import ml_dtypes

    bf16 = ml_dtypes.bfloat16
    x = np.asarray(hidden_states, dtype=np.float32).reshape(T, HID)
    xT = np.ascontiguousarray(x.T).astype(bf16)

    pos = np.arange(T, dtype=np.float32)
    inv_freq = ROPE_BASE ** (-np.arange(0, D, 2, dtype=np.float32) / D)  # [64]
    ang = pos[:, None] * inv_freq[None, :]  # [T, 64]
    cosT = np.cos(ang).T.astype(np.float32)  # [64, T]
    sinT = np.sin(ang).T.astype(np.float32)
    cos2 = np.concatenate([cosT, cosT], axis=0)
    sinS = np.concatenate([-sinT, sinT], axis=0)
    perm = _rope_perm()
    cos2 = np.ascontiguousarray(cos2[perm])
    sinS = np.ascontiguousarray(sinS[perm])

    p = np.arange(128)[:, None]
    tp = np.arange(128)[None, :]
    mask128 = np.ascontiguousarray((p <= tp).astype(np.float16))
    ones = np.ones((128, 128), dtype=np.float16)
    ident = np.eye(128, dtype=np.float16)

    Wq = np.asarray(Wq, dtype=np.float32)
    Wk = np.asarray(Wk, dtype=np.float32)
    Wv = np.asarray(Wv, dtype=np.float32)
    Wo = np.asarray(Wo, dtype=np.float32)

    in_maps = []
    for c in range(N_CORES):
        qs = slice(HQ * D * c, HQ * D * (c + 1))
        ks = slice(D * c, D * (c + 1))
        # permute the head-dim rows of each q head and of k (not v) so the
        # kernel's stream_shuffle rope swap is quadrant-local
        wq = Wq[qs, :].reshape(HQ, D, HID)[:, perm, :].reshape(HQ * D, HID)
        wk = Wk[ks, :][perm, :]
        in_maps.append(
            {
                "xT": xT,
                "wqkvT": np.ascontiguousarray(
                    np.concatenate([wq.T, wk.T, Wv[ks, :].T], axis=1)
                ).astype(bf16),
                "woT": np.ascontiguousarray(Wo[qs, :].T).astype(bf16),
                "cos2": cos2.astype(bf16),
                "sinS": sinS.astype(bf16),
                "masks": mask128,
                "ones_i": ones,
                "ident_i": ident,
            }
        )
    return in_maps


def get_program():
    global _cached
    if _cached is None:
        _cached = _build()
    return _cached


def kernel(hidden_states, Wq, Wk, Wv, Wo):
    nc = get_program()
    in_maps = _host_inputs(hidden_states, Wq, Wk, Wv, Wo)
    res = run_bass_kernel_spmd(nc, in_maps, list(range(N_CORES)))
    outT = np.concatenate([res.results[c]["out"] for c in range(N_CORES)], axis=0)
    return np.ascontiguousarray(outT.T).reshape(1, T, HID).astype(np.float32)
